# revision 14
# baseline (speedup 1.0000x reference)
"""Trainium2 Bass kernel for NeuralODEMemory (nn_NeuralODEMemory_28355374088720).

Math (reference):
    dt = 0.1, 10 Euler steps over h (N=65536 rows, D=512):
        z = [h, t]                              # time feature column
        deriv = tanh(tanh(z @ W1.T + b1) @ W2.T + b2)
        h <- h + dt * deriv
    gate  = sigmoid([x, h10] @ Wg.T + bg)
    out   = gate * h10 + (1 - gate) * x

Kernel strategy (MODE="mir", the default — ~770us; z1 fallback ~1017us):
  * See _build_mir's docstring for the winning mirror-fp8/fp16 design:
    jointly PE/ACT-bound, rb=2048 blocks, 2-slot PSUM rotation, fp16
    h'/d/gate (bf16 h' fails the 2e-2 gate: 2.5e-2; fp16 gives 1.02e-2),
    fp8 DR matmuls for the ODE, fp16 gate, half-block gate pieces popped
    into the next pair's step loop.

Older z1 strategy notes (MODE="z1"):
  * Data-parallel over 8 NeuronCores (8192 rows each); weights replicated.
  * Feature-major ("transposed") activation layout [D, rows] on chip so the
    per-step matmul chain needs no transposes: weights are the stationary
    operand ([in,out] chunks), activations stream as the moving operand, and
    each layer's PSUM output is already in the layout the next layer consumes.
  * The time-feature column is algebraically folded into a per-step bias:
    z @ W1.T = h @ W1[:, :D].T + t * W1[:, D], so b1_eff(s) = b1 + t_s*W1[:,D].
    Bias-add (and input scaling) is free via ACT: out = f(in*scale + bias).
  * h is kept "primed" as h' = h/dt with W1/WgB pre-scaled by dt on the host,
    so the per-step update is a single tensor_tensor add: h' += deriv.
  * z1-accumulation (matmul linearity): z1 = W1s @ h'_s is kept in fp32 SBUF
    and updated with z1 += W1s @ d_s, where d_s = layer-2's tanh output
    written by ACT directly in fp8 — the DoubleRow fp8 matmuls (full 2x MACs
    per instruction, LDWEIGHTS hidden) consume the previous step's ACT
    output with no elementwise op on the critical path.  The gate stays
    bf16 (fp8 gate operands alone cost 1.3-1.8e-2 rel err; see
    quantstudy.py).  Overall rel err ~1.1e-2 vs the 2e-2 gate.
  * Two row-blocks are software-pipelined with a phase-split issue order
    (A1 B1 A2 B2 per step) so the inter-engine dependency cycle
    (a-ACT -> L2 -> d-ACT -> L1delta -> z1-drain -> a-ACT) of one block is
    hidden under the other block's engine work.  GpSimd absorbs group B's
    h-adds; everything else elementwise is merged into large contiguous DVE
    ops.  Each pair's gate is split into per-chunk pieces issued one per
    step inside the NEXT pair's loop -- dependency-free filler the engines
    execute during their per-step waits (engines drain queues in order, so
    only ready work placed ahead of a stalled instruction can fill a gap).
  * MODE="bf16" (plain bf16 matmuls) and MODE="fp8" (the older h-mirror fp8
    design) are kept for reference/fallback via NODE_KERNEL_MODE.
  * Host does the cheap prep: weight transpose/scale/cast, x/h transposes.
"""

import os
from contextlib import ExitStack

import numpy as np
import ml_dtypes

N_TOTAL = 65536
D = 512
NCORES = 8
NPC = N_TOTAL // NCORES          # rows per core
NUM_STEPS = 10
TIME_INTERVAL = 1.0
DT = TIME_INTERVAL / NUM_STEPS
P = 128
FK = D // P                      # feature chunks of 128 (4)
MMN = 512                        # matmul moving-operand free dim (one PSUM bank)

MODE = os.environ.get("NODE_KERNEL_MODE", "mir")

# fp8 (e4m3) power-of-2 scale factors
AH = 4.0                         # h' mirror scale
AW1 = 2.0 ** 15                  # on dt*W1s  (|dt*W1s| <= 0.0044 -> <= 145)
AW2 = 2.0 ** 12                  # on W2      (|W2| <= 0.0442 -> <= 181)
AX = 32.0                        # x mirror   (|x| <= ~5.3 -> <= 170)
CG = 2.0 ** 16                   # common gate scale: awga*AX = awgb*AH = CG
AWGA = CG / AX                   # 2048  (|WgA| <= 0.0313 -> <= 64)
AWGB = CG / AH                   # 16384 (|dt*WgB| <= 0.0032 -> <= 52)

_CACHE = {}
LAST = {}                        # stash of the last run's BassKernelResults


def _block_schedule(npc, rblk):
    # Uniform blocks: lead blocks smaller than rblk were tried to shorten the
    # serial head, but 512-row blocks starve the PE mid-kernel (PSUM slots and
    # short matmul groups can't hide the ACT/DVE tails) — a net loss.
    return [(i * rblk, rblk) for i in range(npc // rblk)]


def _build(npc, rblk=1024, mode=MODE):
    import concourse.mybir as mybir
    import concourse.tile as tile
    from concourse import bacc

    f32 = mybir.dt.float32
    bf16 = mybir.dt.bfloat16
    fp8 = mybir.dt.float8e4
    Act = mybir.ActivationFunctionType
    Alu = mybir.AluOpType
    DR = mybir.MatmulPerfMode.DoubleRow

    # per-tensor matmul dtypes: "fp8" = everything, "fp8l2" = only layer 2
    # (a and W2 — both well-conditioned for e4m3), else bf16
    cdt = fp8 if mode == "fp8" else bf16          # h-mirror / x-mirror / W1 / Wg
    l2dt = fp8 if mode in ("fp8", "fp8l2") else bf16   # a / W2
    dr1 = mode == "fp8"                            # DoubleRow on layer 1 + gate
    dr2 = mode in ("fp8", "fp8l2")                 # DoubleRow on layer 2
    blocks = _block_schedule(npc, rblk)

    nc = bacc.Bacc("TRN2", target_bir_lowering=False, debug=False,
                   num_devices=NCORES)

    hT = nc.dram_tensor("hT", [D, npc], f32, kind="ExternalInput").ap()
    xT = nc.dram_tensor("xT", [D, npc], f32, kind="ExternalInput").ap()
    xTb = nc.dram_tensor("xTb", [D, npc], cdt, kind="ExternalInput").ap()
    w1t = nc.dram_tensor("w1t", [D, D], cdt, kind="ExternalInput").ap()
    w2t = nc.dram_tensor("w2t", [D, D], l2dt, kind="ExternalInput").ap()
    wgt = nc.dram_tensor("wgt", [2 * D, D], cdt, kind="ExternalInput").ap()
    b1e = nc.dram_tensor("b1e", [P, NUM_STEPS * FK], f32, kind="ExternalInput").ap()
    b2c = nc.dram_tensor("b2c", [P, FK], f32, kind="ExternalInput").ap()
    bgc = nc.dram_tensor("bgc", [P, FK], f32, kind="ExternalInput").ap()
    outT = nc.dram_tensor("outT", [D, npc], fp16, kind="ExternalOutput").ap()

    hTr = hT.rearrange("(k p) r -> p k r", p=P)
    xTr = xT.rearrange("(k p) r -> p k r", p=P)
    xTbr = xTb.rearrange("(k p) r -> p k r", p=P)
    outTr = outT.rearrange("(k p) r -> p k r", p=P)

    # ACT input scales that undo the fp8 operand scaling exactly
    s_l1 = 1.0 / (AW1 * AH) if dr1 else 1.0
    s_l2 = 1.0 / AW2 if dr2 else 1.0
    s_g = 1.0 / CG if dr1 else 1.0

    with tile.TileContext(nc) as tc:
        with ExitStack() as ctx:
            consts = ctx.enter_context(tc.tile_pool(name="consts", bufs=1))
            hp = ctx.enter_context(tc.tile_pool(name="h", bufs=4))
            hbp = ctx.enter_context(tc.tile_pool(name="hb", bufs=2))
            apool = ctx.enter_context(tc.tile_pool(name="a", bufs=2))
            dpool = ctx.enter_context(tc.tile_pool(name="d", bufs=2))
            xbp = ctx.enter_context(tc.tile_pool(name="xb", bufs=4))
            xfp = ctx.enter_context(tc.tile_pool(name="xf", bufs=2))
            gp = ctx.enter_context(tc.tile_pool(name="g", bufs=2))
            scp = ctx.enter_context(tc.tile_pool(name="sc", bufs=6))
            psp = ctx.enter_context(tc.tile_pool(name="ps", bufs=4, space="PSUM"))

            # Replicated constants, resident for the whole kernel.
            w1 = consts.tile([P, FK, D], cdt)
            nc.sync.dma_start(w1[:], w1t.rearrange("(k p) m -> p k m", p=P))
            w2 = consts.tile([P, FK, D], l2dt)
            nc.sync.dma_start(w2[:], w2t.rearrange("(k p) m -> p k m", p=P))
            wg = consts.tile([P, 2 * FK, D], cdt)
            nc.sync.dma_start(wg[:], wgt.rearrange("(k p) m -> p k m", p=P))
            b1 = consts.tile([P, NUM_STEPS * FK], f32)
            nc.sync.dma_start(b1[:], b1e)
            b2 = consts.tile([P, FK], f32)
            nc.sync.dma_start(b2[:], b2c)
            bg = consts.tile([P, FK], f32)
            nc.sync.dma_start(bg[:], bgc)

            def mm_group(ps_tile, wtile, wk, m, rhs_tile, rk, start, stop,
                         nsub, dr):
                for sub in range(nsub):
                    if dr:
                        # DoubleRow: one matmul contracts a pair of k-chunks
                        nc.tensor.matmul(
                            ps_tile[:, sub * MMN:(sub + 1) * MMN],
                            wtile[:, 2 * wk:2 * wk + 2, m * P:(m + 1) * P],
                            rhs_tile[:, 2 * rk:2 * rk + 2,
                                     sub * MMN:(sub + 1) * MMN],
                            start=start, stop=stop, perf_mode=DR,
                        )
                    else:
                        nc.tensor.matmul(
                            ps_tile[:, sub * MMN:(sub + 1) * MMN],
                            wtile[:, wk, m * P:(m + 1) * P],
                            rhs_tile[:, rk, sub * MMN:(sub + 1) * MMN],
                            start=start, stop=stop,
                        )

            NK1 = FK // 2 if dr1 else FK      # layer-1 / gate k-group count
            NK2 = FK // 2 if dr2 else FK      # layer-2 k-group count

            def mirror(dst, src):
                # bf16/fp8 matmul mirror of the fp32 h' accumulator
                if mode == "fp8":
                    nc.vector.tensor_scalar_mul(dst, src, AH)
                else:
                    nc.vector.tensor_copy(dst, src)

            for rs, rblk_b in blocks:
                nsub = rblk_b // MMN
                h = hp.tile([P, FK, rblk_b], f32, tag="h")
                # per-chunk loads so each mirror cast starts as soon as its
                # chunk lands (matters for the first block's serial head)
                for k in range(FK):
                    nc.sync.dma_start(h[:, k], hTr[:, k, rs:rs + rblk_b])
                xb = xbp.tile([P, FK, rblk_b], cdt, tag="xb")
                nc.sync.dma_start(xb[:], xTbr[:, :, rs:rs + rblk_b])
                xf = xfp.tile([P, FK, rblk_b], f32, tag="xf")
                nc.sync.dma_start(xf[:], xTr[:, :, rs:rs + rblk_b])

                hb = hbp.tile([P, FK, rblk_b], cdt, tag="hb")
                for k in range(FK):
                    mirror(hb[:, k], h[:, k])

                for s in range(NUM_STEPS):
                    # layer 1: a = tanh((dt*W1s).T-chunks @ h'b + b1_eff(s))
                    a = apool.tile([P, FK, rblk_b], l2dt, tag="a")
                    for m in range(FK):
                        ps = psp.tile([P, rblk_b], f32, tag="ps")
                        for k in range(NK1):
                            mm_group(ps, w1, k, m, hb, k, k == 0, k == NK1 - 1,
                                     nsub, dr1)
                        col = s * FK + m
                        nc.scalar.activation(a[:, m], ps[:], Act.Tanh,
                                             bias=b1[:, col:col + 1], scale=s_l1)
                    # layer 2: deriv = tanh(W2.T-chunks @ a + b2)
                    d = dpool.tile([P, FK, rblk_b], bf16, tag="d")
                    for m in range(FK):
                        ps = psp.tile([P, rblk_b], f32, tag="ps")
                        for k in range(NK2):
                            mm_group(ps, w2, k, m, a, k, k == 0, k == NK2 - 1,
                                     nsub, dr2)
                        nc.scalar.activation(d[:, m], ps[:], Act.Tanh,
                                             bias=b2[:, m:m + 1], scale=s_l2)
                    # h' += deriv ; refresh matmul mirror.  Split the fp32
                    # adds across DVE and GpSimd to balance engine load.
                    hb = hbp.tile([P, FK, rblk_b], cdt, tag="hb")
                    for k in range(FK):
                        eng = nc.vector if (mode != "fp8" or k < 2) else nc.gpsimd
                        eng.tensor_tensor(h[:, k], h[:, k], d[:, k], Alu.add)
                        mirror(hb[:, k], h[:, k])

                # gate + combine, per output feature chunk
                for m in range(FK):
                    ps = psp.tile([P, rblk_b], f32, tag="ps")
                    for k in range(NK1):
                        mm_group(ps, wg, k, m, xb, k, k == 0, False, nsub, dr1)
                    for k in range(NK1):
                        mm_group(ps, wg, NK1 + k, m, hb, k, False,
                                 k == NK1 - 1, nsub, dr1)
                    g = gp.tile([P, rblk_b], f32, tag="g")
                    nc.scalar.activation(g[:], ps[:], Act.Sigmoid,
                                         bias=bg[:, m:m + 1], scale=s_g)
                    # out = x + g * (dt*h' - x)
                    dif = scp.tile([P, rblk_b], f32, tag="sc")
                    nc.vector.scalar_tensor_tensor(
                        dif[:], h[:, m], float(DT), xf[:, m],
                        Alu.mult, Alu.subtract)
                    gd = scp.tile([P, rblk_b], f32, tag="sc")
                    nc.vector.tensor_tensor(gd[:], g[:], dif[:], Alu.mult)
                    ot = scp.tile([P, rblk_b], f32, tag="sc")
                    nc.vector.tensor_tensor(ot[:], xf[:, m], gd[:], Alu.add)
                    nc.sync.dma_start(outTr[:, m, rs:rs + rblk_b], ot[:])

    nc.compile()
    return nc


def _build_z1(npc, rblk=1024):
    """z1-accumulation architecture, fp8 ODE loop + bf16 gate (v4).

    Math: keep z1 = (AW1*dt*W1) @ h'_s in fp32 SBUF, update z1 += W1s @ d_s
    where d_s is layer-2's fp8 ACT output (matmul linearity) -- no per-step
    h-mirror.  h' (fp32) accumulates d_s off the critical path.

    Scheduling: the per-step dependency cycle (a-ACT -> L2 MMs -> d-ACT ->
    L1delta MMs -> z1 drain -> next a-ACT) is ~2x longer than any engine's
    per-step work, so TWO adjacent row blocks are SOFTWARE-PIPELINED: all
    engines fill group A's chain gaps with group B's work.  With that slack,
    GpSimd (slow but otherwise idle) safely absorbs group B's h-adds and the
    gate's gd/ot products.  DVE ops are merged into big contiguous APs
    ([P, 2, rblk] psum pair drains, [P, FK*rblk] h-adds).
    Accuracy (CPU sim ~1.14e-2, HW ~1.09e-2): fp8 bounds from quantstudy.py
    -- gate must stay bf16, h' must stay fp32.
    """
    import concourse.mybir as mybir
    import concourse.tile as tile
    from concourse import bacc

    f32 = mybir.dt.float32
    bf16 = mybir.dt.bfloat16
    fp8 = mybir.dt.float8e4
    Act = mybir.ActivationFunctionType
    Alu = mybir.AluOpType
    DR = mybir.MatmulPerfMode.DoubleRow

    blocks = _block_schedule(npc, rblk)
    pairs = [(blocks[i], blocks[i + 1]) for i in range(0, len(blocks), 2)]

    nc = bacc.Bacc("TRN2", target_bir_lowering=False, debug=False,
                   num_devices=NCORES)

    hT = nc.dram_tensor("hT", [D, npc], f32, kind="ExternalInput").ap()
    h0b = nc.dram_tensor("h0b", [D, npc], fp8, kind="ExternalInput").ap()
    xTb = nc.dram_tensor("xTb", [D, npc], bf16, kind="ExternalInput").ap()
    w1t = nc.dram_tensor("w1t", [D, D], fp8, kind="ExternalInput").ap()
    w2t = nc.dram_tensor("w2t", [D, D], fp8, kind="ExternalInput").ap()
    wgt = nc.dram_tensor("wgt", [2 * D, D], bf16, kind="ExternalInput").ap()
    b1e = nc.dram_tensor("b1e", [P, NUM_STEPS * FK], f32, kind="ExternalInput").ap()
    b2c = nc.dram_tensor("b2c", [P, FK], f32, kind="ExternalInput").ap()
    bgc = nc.dram_tensor("bgc", [P, FK], f32, kind="ExternalInput").ap()
    outT = nc.dram_tensor("outT", [D, npc], f32, kind="ExternalOutput").ap()

    hTr = hT.rearrange("(k p) r -> p k r", p=P)
    h0br = h0b.rearrange("(k p) r -> p k r", p=P)
    xTbr = xTb.rearrange("(k p) r -> p k r", p=P)
    outTr = outT.rearrange("(k p) r -> p k r", p=P)

    s_l1 = 1.0 / AW1
    s_l2 = 1.0 / AW2
    NKG = FK // 2                 # DR k-group count (2)

    with tile.TileContext(nc) as tc:
        with ExitStack() as ctx:
            consts = ctx.enter_context(tc.tile_pool(name="consts", bufs=1))
            z1p = ctx.enter_context(tc.tile_pool(name="z1", bufs=2))
            hp = ctx.enter_context(tc.tile_pool(name="h", bufs=4))
            hb0p = ctx.enter_context(tc.tile_pool(name="hb0", bufs=1))
            apool = ctx.enter_context(tc.tile_pool(name="a", bufs=2))
            dpool = ctx.enter_context(tc.tile_pool(name="d", bufs=4))
            xbp = ctx.enter_context(tc.tile_pool(name="xb", bufs=4))
            hgp = ctx.enter_context(tc.tile_pool(name="hg", bufs=2))
            gp = ctx.enter_context(tc.tile_pool(name="g", bufs=1))
            scp = ctx.enter_context(tc.tile_pool(name="sc", bufs=4))
            psp = ctx.enter_context(tc.tile_pool(name="ps", bufs=4, space="PSUM"))

            # DMA order matters for the serial head: the first L1 fill
            # needs only w1 + hb0(block A) + b1, so those go first; the
            # gate-only constants (wg, bg) queue last.
            w1 = consts.tile([P, FK, D], fp8)
            nc.sync.dma_start(w1[:], w1t.rearrange("(k p) m -> p k m", p=P))
            b1 = consts.tile([P, NUM_STEPS * FK], f32)
            nc.sync.dma_start(b1[:], b1e)
            w2 = consts.tile([P, FK, D], fp8)
            nc.sync.dma_start(w2[:], w2t.rearrange("(k p) m -> p k m", p=P))
            b2 = consts.tile([P, FK], f32)
            nc.sync.dma_start(b2[:], b2c)
            wg = consts.tile([P, 2 * FK, D], fp16)
            nc.sync.dma_start(wg[:], wgt.rearrange("(k p) m -> p k m", p=P))
            bg = consts.tile([P, FK], f32)
            nc.sync.dma_start(bg[:], bgc)

            def dr_one(ps, wtile, m, rhs, nsub):
                # fill psum tile ps[P, rblk] with output chunk m
                for kg in range(NKG):
                    for sub in range(nsub):
                        nc.tensor.matmul(
                            ps[:, sub * MMN:(sub + 1) * MMN],
                            wtile[:, 2 * kg:2 * kg + 2, m * P:(m + 1) * P],
                            rhs[:, 2 * kg:2 * kg + 2,
                                sub * MMN:(sub + 1) * MMN],
                            start=kg == 0, stop=kg == NKG - 1, perf_mode=DR,
                        )

            def load_group(rs, rb, gi):
                h = hp.tile([P, FK, rb], f32, tag="h", name=f"h{gi}")
                for k in range(FK):
                    nc.sync.dma_start(h[:, k], hTr[:, k, rs:rs + rb])
                hb0 = hb0p.tile([P, FK, rb], fp8, tag="hb0", name=f"hb0{gi}")
                nc.sync.dma_start(hb0[:], h0br[:, :, rs:rs + rb])
                xb = xbp.tile([P, FK, rb], bf16, tag="xb", name=f"xb{gi}")
                nc.sync.dma_start(xb[:], xTbr[:, :, rs:rs + rb])
                return h, hb0, xb

            def init_group(hb0, rb, gi):
                nsub = rb // MMN
                z1 = z1p.tile([P, FK, rb], f32, tag="z1", name=f"z1{gi}")
                for m in range(FK):
                    ps = psp.tile([P, rb], f32, tag="ps", name=f"psi{gi}")
                    dr_one(ps, w1, m, hb0, nsub)
                    nc.vector.tensor_copy(z1[:, m], ps[:])
                return z1

            def step_part1(s, z1, rb, gi):
                # a = tanh(z1/AW1 + b1_eff(s)) then d = tanh(W2 @ a/AW2 + b2)
                nsub = rb // MMN
                a = apool.tile([P, FK, rb], fp8, tag="a", name=f"a{gi}")
                for m in range(FK):
                    col = s * FK + m
                    nc.scalar.activation(a[:, m], z1[:, m], Act.Tanh,
                                         bias=b1[:, col:col + 1], scale=s_l1)
                d = dpool.tile([P, FK, rb], fp8, tag="d", name=f"d{gi}")
                for m in range(FK):
                    ps = psp.tile([P, rb], f32, tag="ps", name=f"ps2{gi}")
                    dr_one(ps, w2, m, a, nsub)
                    nc.scalar.activation(d[:, m], ps[:], Act.Tanh,
                                         bias=b2[:, m:m + 1], scale=s_l2)
                return d

            def step_part2(s, z1, h, d, rb, gi):
                # z1 += W1s @ d first (gates next step's a-ACT), then h' += d
                # (merged; group B rides GpSimd except the last step, whose
                # h-add gates the gate phase)
                nsub = rb // MMN
                if s < NUM_STEPS - 1:
                    for m in range(FK):
                        ps = psp.tile([P, rb], f32, tag="ps", name=f"ps1{gi}")
                        dr_one(ps, w1, m, d, nsub)
                        nc.vector.tensor_tensor(z1[:, m], z1[:, m],
                                                ps[:], Alu.add)
                heng = nc.gpsimd if (gi == 1 and s < NUM_STEPS - 1) else nc.vector
                heng.tensor_tensor(h[:], h[:], d[:], Alu.add)

            def gate_prep(h, gi):
                # bf16 mirror of h' for the gate matmul (after the last h-add)
                rb = h.shape[2]
                hbg = hgp.tile([P, FK, rb], bf16, tag="hbg", name=f"hbg{gi}")
                nc.vector.tensor_copy(hbg[:], h[:])
                return hbg

            def gate_piece(h, xb, hbg, rs, rb, gi, m):
                # one output chunk of the gate: issued as ready filler work
                # inside the NEXT pair's step loop
                nsub = rb // MMN
                if True:
                    ps = psp.tile([P, rb], f32, tag="ps", name=f"psg{gi}")
                    for sub in range(nsub):
                        sl = slice(sub * MMN, (sub + 1) * MMN)
                        for k in range(FK):
                            nc.tensor.matmul(
                                ps[:, sl], wg[:, k, m * P:(m + 1) * P],
                                xb[:, k, sl], start=k == 0, stop=False)
                        for k in range(FK):
                            nc.tensor.matmul(
                                ps[:, sl],
                                wg[:, FK + k, m * P:(m + 1) * P],
                                hbg[:, k, sl], start=False,
                                stop=k == FK - 1)
                    if True:
                        g = gp.tile([P, rb], f32, tag="g", name=f"g{gi}")
                        nc.scalar.activation(g[:], ps[:], Act.Sigmoid,
                                             bias=bg[:, m:m + 1], scale=1.0)
                        # out = x + g*(dt*h' - x), x read as bf16 (xb)
                        dif = scp.tile([P, rb], f32, tag="sc", name=f"dif{gi}")
                        nc.vector.scalar_tensor_tensor(
                            dif[:], h[:, m], float(DT), xb[:, m],
                            Alu.mult, Alu.subtract)
                        gd = scp.tile([P, rb], f32, tag="sc", name=f"gd{gi}")
                        nc.vector.tensor_tensor(gd[:], g[:], dif[:], Alu.mult)
                        ot = scp.tile([P, rb], f32, tag="sc", name=f"ot{gi}")
                        nc.vector.tensor_tensor(ot[:], xb[:, m], gd[:], Alu.add)
                        nc.sync.dma_start(outTr[:, m, rs:rs + rb], ot[:])

            pend = []
            for (rsA, rbA), (rsB, rbB) in pairs:
                hA, hb0A, xbA = load_group(rsA, rbA, 0)
                hB, hb0B, xbB = load_group(rsB, rbB, 1)
                z1A = init_group(hb0A, rbA, 0)
                z1B = init_group(hb0B, rbB, 1)
                for s in range(NUM_STEPS):
                    dA = step_part1(s, z1A, rbA, 0)
                    dB = step_part1(s, z1B, rbB, 1)
                    step_part2(s, z1A, hA, dA, rbA, 0)
                    step_part2(s, z1B, hB, dB, rbB, 1)
                    if pend:
                        pend.pop(0)()
                # previous pair's gate pieces not yet drained (first 2 steps
                # of this pair had none queued yet on the very first pair)
                while pend:
                    pend.pop(0)()
                hbgA = gate_prep(hA, 0)
                hbgB = gate_prep(hB, 1)
                pend = [
                    (lambda h=h_, xb=xb_, hbg=hbg_, rs=rs_, rb=rb_, gi=gi_,
                            m=m_: gate_piece(h, xb, hbg, rs, rb, gi, m))
                    for (h_, xb_, hbg_, rs_, rb_, gi_) in
                    ((hA, xbA, hbgA, rsA, rbA, 0), (hB, xbB, hbgB, rsB, rbB, 1))
                    for m_ in range(FK)
                ]
            while pend:
                pend.pop(0)()

    nc.compile()
    return nc


def _build_mir(npc, rb=2048):
    """Mirror-fp8 architecture (v5) — no z1 accumulator, ACT-bound pipeline.

    Per step (per 2048-row block):
        L1 psum = (AW1*dt*W1).T @ hb      fp8 DR matmuls, 8 insts per m
        a  = tanh(psum/AW1 + b1e(s,m))    ACT psum->SBUF fp8, FD=2048
        L2 psum = (AW2*W2).T @ a          fp8 DR
        d  = tanh(psum/AW2 + b2)          ACT psum->SBUF bf16
        h += d                            DVE bf16 TT (2x mode), per k-chunk
        hb = fp8(h)                       DVE copy-cast, per k-pair
    h' = h/dt is kept in bf16 (quantstudy2: adds ~1e-3 vs fp32; total rel
    err ~1.0-1.2e-2 incl. fp8 W1/W2/a/hb quantization).  The gate matmul
    reads h (bf16) directly — no separate bf16 gate mirror.

    Why this beats z1 (1017us): microbench (bench1.py) showed matmuls
    sustain 216ns (LDWEIGHTS fully hidden even with changing stationary)
    and ACT calls carry a fixed ~360ns overhead, so the kernel is jointly
    PE/ACT-bound at ~676/683us with FD=2048 ACT drains.  z1's fp32 psum
    drains (390us DVE) are replaced by cheap bf16 adds + fp8 casts, and
    rb=2048 halves the ACT call count vs rb=1024.

    PSUM: 2 slots of [P, 2048] f32 (4 banks each) ping-ponged through one
    pool tag; ACT drains a whole m-chunk per call.  Two blocks run their
    10 steps together (pair); the previous pair's gate pieces + xb loads
    pop one-per-step inside the next pair's loop as PE/ACT filler.
    """
    import concourse.mybir as mybir
    import concourse.tile as tile
    from concourse import bacc

    f32 = mybir.dt.float32
    fp16 = mybir.dt.float16
    fp8 = mybir.dt.float8e4
    Act = mybir.ActivationFunctionType
    Alu = mybir.AluOpType
    DR = mybir.MatmulPerfMode.DoubleRow

    nsub = rb // MMN
    nblk = npc // rb
    assert nblk % 2 == 0
    pairs = [(2 * i, 2 * i + 1) for i in range(nblk // 2)]

    nc = bacc.Bacc("TRN2", target_bir_lowering=False, debug=False,
                   num_devices=NCORES)

    hT = nc.dram_tensor("hT", [D, npc], fp16, kind="ExternalInput").ap()
    h0b = nc.dram_tensor("h0b", [D, npc], fp8, kind="ExternalInput").ap()
    xTb = nc.dram_tensor("xTb", [D, npc], fp16, kind="ExternalInput").ap()
    w1t = nc.dram_tensor("w1t", [D, D], fp8, kind="ExternalInput").ap()
    w2t = nc.dram_tensor("w2t", [D, D], fp8, kind="ExternalInput").ap()
    wgt = nc.dram_tensor("wgt", [2 * D, D], fp16, kind="ExternalInput").ap()
    b1e = nc.dram_tensor("b1e", [P, NUM_STEPS * FK], f32, kind="ExternalInput").ap()
    b2c = nc.dram_tensor("b2c", [P, FK], f32, kind="ExternalInput").ap()
    bgc = nc.dram_tensor("bgc", [P, FK], f32, kind="ExternalInput").ap()
    outT = nc.dram_tensor("outT", [D, npc], f32, kind="ExternalOutput").ap()

    hTr = hT.rearrange("(k p) r -> p k r", p=P)
    h0br = h0b.rearrange("(k p) r -> p k r", p=P)
    xTbr = xTb.rearrange("(k p) r -> p k r", p=P)
    outTr = outT.rearrange("(k p) r -> p k r", p=P)

    s_l1 = 1.0 / AW1
    s_l2 = 1.0 / AW2

    with tile.TileContext(nc) as tc:
        with ExitStack() as ctx:
            consts = ctx.enter_context(tc.tile_pool(name="consts", bufs=1))
            hp = ctx.enter_context(tc.tile_pool(name="h", bufs=4))
            hbp = ctx.enter_context(tc.tile_pool(name="hb", bufs=3))
            apool = ctx.enter_context(tc.tile_pool(name="a", bufs=2))
            dpool = ctx.enter_context(tc.tile_pool(name="d", bufs=2))
            xbp = ctx.enter_context(tc.tile_pool(name="xb", bufs=2))
            gp = ctx.enter_context(tc.tile_pool(name="g", bufs=1))
            scp = ctx.enter_context(tc.tile_pool(name="sc", bufs=2))
            otp = ctx.enter_context(tc.tile_pool(name="ot", bufs=2))
            psp = ctx.enter_context(tc.tile_pool(name="ps", bufs=2, space="PSUM"))

            w1 = consts.tile([P, FK, D], fp8)
            nc.sync.dma_start(w1[:], w1t.rearrange("(k p) m -> p k m", p=P))
            w2 = consts.tile([P, FK, D], fp8)
            nc.sync.dma_start(w2[:], w2t.rearrange("(k p) m -> p k m", p=P))
            wg = consts.tile([P, 2 * FK, D], bf16)
            nc.sync.dma_start(wg[:], wgt.rearrange("(k p) m -> p k m", p=P))
            b1 = consts.tile([P, NUM_STEPS * FK], f32)
            nc.sync.dma_start(b1[:], b1e)
            b2 = consts.tile([P, FK], f32)
            nc.sync.dma_start(b2[:], b2c)
            bg = consts.tile([P, FK], f32)
            nc.sync.dma_start(bg[:], bgc)

            # First ACT instruction is a sigmoid: loads the sigmoid table
            # set (which also contains tanh) once, during the DMA head,
            # instead of a tanh-set load now plus a sigmoid-set switch
            # stalling the ACT stream at the first gate piece.
            warm = consts.tile([P, 1], f32)
            nc.scalar.activation(warm[:], b2[:, 0:1],
                                 Act.Sigmoid, bias=0.0, scale=1.0)

            def dr_fill(ps, wtile, m, rhs):
                # one m-chunk of a 512-contract fp8 DR matmul group
                for kg in range(FK // 2):
                    for sub in range(nsub):
                        nc.tensor.matmul(
                            ps[:, sub * MMN:(sub + 1) * MMN],
                            wtile[:, 2 * kg:2 * kg + 2, m * P:(m + 1) * P],
                            rhs[:, 2 * kg:2 * kg + 2,
                                sub * MMN:(sub + 1) * MMN],
                            start=kg == 0, stop=kg == FK // 2 - 1,
                            perf_mode=DR,
                        )

            def load_block(rs, b):
                hb = hbp.tile([P, FK, rb], fp8, tag="hb", name=f"hb{b}")
                nc.sync.dma_start(hb[:, 0:2], h0br[:, 0:2, rs:rs + rb])
                nc.sync.dma_start(hb[:, 2:4], h0br[:, 2:4, rs:rs + rb])
                h = hp.tile([P, FK, rb], fp16, tag="h", name=f"h{b}")
                for k in range(FK):
                    nc.sync.dma_start(h[:, k], hTr[:, k, rs:rs + rb])
                return h, hb

            def l1_phase(s, b, hb):
                a = apool.tile([P, FK, rb], fp8, tag="a", name=f"a{b}")
                for m in range(FK):
                    ps = psp.tile([P, rb], f32, tag="ps", name=f"ps1{b}")
                    dr_fill(ps, w1, m, hb)
                    col = s * FK + m
                    nc.scalar.activation(a[:, m], ps[:], Act.Tanh,
                                         bias=b1[:, col:col + 1], scale=s_l1)
                return a

            def l2_phase(s, b, a, h):
                # L2 matmuls + d-ACT, then h += d per chunk and the fp8
                # re-mirror per chunk-pair (skipped on the last step)
                d = dpool.tile([P, FK, rb], fp16, tag="d", name=f"d{b}")
                hbn = None
                if s < NUM_STEPS - 1:
                    hbn = hbp.tile([P, FK, rb], fp8, tag="hb", name=f"hbn{b}")
                for m in range(FK):
                    ps = psp.tile([P, rb], f32, tag="ps", name=f"ps2{b}")
                    dr_fill(ps, w2, m, a)
                    nc.scalar.activation(d[:, m], ps[:], Act.Tanh,
                                         bias=b2[:, m:m + 1], scale=s_l2)
                    nc.vector.tensor_tensor(h[:, m], h[:, m], d[:, m], Alu.add)
                    if hbn is not None and m % 2 == 1:
                        kg = m // 2
                        nc.vector.tensor_copy(hbn[:, 2 * kg:2 * kg + 2],
                                              h[:, 2 * kg:2 * kg + 2])
                return hbn

            def load_xb(rs, b):
                xb = xbp.tile([P, FK, rb], fp16, tag="xb", name=f"xb{b}")
                nc.sync.dma_start(xb[:], xTbr[:, :, rs:rs + rb])
                return xb

            def gate_piece(h, xb, rs, b, m, ro, w):
                # Gate piece over rows [ro, ro+w) of the block.  ODE pops use
                # w=512 (an 8-inst fill shorter than one a/d-ACT drain, so the
                # 2-slot psum rotation never starves ACT); the tail, which is
                # PE-bound, uses w=1024 for denser fills.
                ps = psp.tile([P, w], f32, tag="ps", name=f"psg{b}")
                for k in range(2 * FK):
                    rhs = xb[:, k] if k < FK else h[:, k - FK]
                    for sub in range(w // MMN):
                        sl = slice(sub * MMN, (sub + 1) * MMN)
                        so = slice(ro + sub * MMN, ro + (sub + 1) * MMN)
                        nc.tensor.matmul(ps[:, sl],
                                         wg[:, k, m * P:(m + 1) * P],
                                         rhs[:, so], start=k == 0,
                                         stop=k == 2 * FK - 1)
                g = gp.tile([P, w], fp16, tag="g", name=f"g{b}")
                nc.scalar.activation(g[:], ps[:], Act.Sigmoid,
                                     bias=bg[:, m:m + 1], scale=1.0)
                # out = xb + g * (dt*h' - xb), fp16 intermediates
                dif = scp.tile([P, w], fp16, tag="sc", name=f"dif{b}")
                nc.vector.scalar_tensor_tensor(dif[:], h[:, m, ro:ro + w],
                                               float(DT), xb[:, m, ro:ro + w],
                                               Alu.mult, Alu.subtract)
                gd = scp.tile([P, w], fp16, tag="sc", name=f"gd{b}")
                nc.vector.tensor_tensor(gd[:], g[:], dif[:], Alu.mult)
                ot = otp.tile([P, w], fp16, tag="ot", name=f"ot{b}")
                nc.vector.tensor_tensor(ot[:], xb[:, m, ro:ro + w],
                                        gd[:], Alu.add)
                nc.sync.dma_start(outTr[:, m, rs + ro:rs + ro + w], ot[:])

            pend = []
            for pi, (bA, bB) in enumerate(pairs):
                last = pi == len(pairs) - 1
                rsA, rsB = bA * rb, bB * rb
                hA, hbA = load_block(rsA, bA)
                hB, hbB = load_block(rsB, bB)
                for s in range(NUM_STEPS):
                    aA = l1_phase(s, bA, hbA)
                    aB = l1_phase(s, bB, hbB)
                    if pend:
                        pend.pop(0)[1]()
                    hbA2 = l2_phase(s, bA, aA, hA)
                    hbB2 = l2_phase(s, bB, aB, hB)
                    hbA, hbB = hbA2, hbB2
                    if pend:
                        pend.pop(0)[1]()
                while pend:
                    pend.pop(0)[1]()
                for h_, rs_, b_ in ((hA, rsA, bA), (hB, rsB, bB)):
                    def mk_load(h=h_, rs=rs_, b=b_):
                        state = {}

                        def go():
                            state["xb"] = load_xb(rs, b)
                        return state, go
                    state, go = mk_load()
                    pend.append(("load", go))
                    w_ = 2048 if last else 1024
                    for m_ in range(FK):
                        for q_ in range(rb // w_):
                            pend.append(
                                ("piece",
                                 lambda h=h_, rs=rs_, b=b_, m=m_,
                                        ro=q_ * w_, w=w_, st=state:
                                 gate_piece(h, st["xb"], rs, b, m, ro, w)))
                if last:
                    # tail: issue the xb DMAs first (xbp is double-buffered)
                    # so the gate pieces never wait on a load
                    pend.sort(key=lambda e: e[0] != "load")
            while pend:
                pend.pop(0)[1]()

    nc.compile()
    return nc


def _get_nc(npc, rblk=1024, mode=MODE):
    key = (npc, rblk, mode)
    if key not in _CACHE:
        if mode == "mir":
            _CACHE[key] = _build_mir(npc)
        elif mode == "z1":
            _CACHE[key] = _build_z1(npc, rblk)
        else:
            _CACHE[key] = _build(npc, rblk, mode)
    return _CACHE[key]


def _fp8_np():
    import concourse.mybir as mybir
    return mybir.dt.np(mybir.dt.float8e4)


def _cast_dt(mode):
    # dtype of the x-mirror / W1 / Wg operands
    return _fp8_np() if mode == "fp8" else ml_dtypes.bfloat16


def _host_prep(W1, b1, W2, b2, Wg, bg, mode=MODE):
    cdt = _cast_dt(mode)
    W1 = np.asarray(W1, np.float32)
    W2 = np.asarray(W2, np.float32)
    Wg = np.asarray(Wg, np.float32)
    b1 = np.asarray(b1, np.float32)
    b2 = np.asarray(b2, np.float32)
    bg = np.asarray(bg, np.float32)

    if mode == "fp8":
        sw1, sw2, swga, swgb = AW1, AW2, AWGA, AWGB
    elif mode == "fp8l2":
        sw1, sw2, swga, swgb = 1.0, AW2, 1.0, 1.0
    else:
        sw1 = sw2 = swga = swgb = 1.0
    l2dt = _fp8_np() if mode in ("fp8", "fp8l2") else cdt

    w1t = np.ascontiguousarray((sw1 * DT * W1[:, :D]).T).astype(cdt)  # [in, out]
    w2t = np.ascontiguousarray((sw2 * W2).T).astype(l2dt)
    wgt = np.ascontiguousarray(
        np.concatenate([swga * Wg[:, :D].T, swgb * DT * Wg[:, D:].T],
                       axis=0)).astype(cdt)

    ts = (DT * np.arange(NUM_STEPS)).astype(np.float32)
    b1r = b1.reshape(FK, P)                                        # [m, p]
    wtr = np.ascontiguousarray(W1[:, D]).reshape(FK, P)            # [m, p]
    b1e = b1r[None, :, :] + ts[:, None, None] * wtr[None, :, :]    # [s, m, p]
    b1e = np.ascontiguousarray(b1e.transpose(2, 0, 1).reshape(P, NUM_STEPS * FK))
    b2c = np.ascontiguousarray(b2.reshape(FK, P).T)
    bgc = np.ascontiguousarray(bg.reshape(FK, P).T)
    return dict(w1t=w1t, w2t=w2t, wgt=wgt,
                b1e=b1e.astype(np.float32),
                b2c=b2c.astype(np.float32), bgc=bgc.astype(np.float32))


def _host_prep_z1(W1, b1, W2, b2, Wg, bg):
    F8 = _fp8_np()
    BF = ml_dtypes.bfloat16
    W1 = np.asarray(W1, np.float32)
    W2 = np.asarray(W2, np.float32)
    Wg = np.asarray(Wg, np.float32)
    b1 = np.asarray(b1, np.float32)
    b2 = np.asarray(b2, np.float32)
    bg = np.asarray(bg, np.float32)

    w1t = np.ascontiguousarray((AW1 * DT * W1[:, :D]).T).astype(F8)  # [in, out]
    w2t = np.ascontiguousarray((AW2 * W2).T).astype(F8)
    wgt = np.ascontiguousarray(
        np.concatenate([Wg[:, :D].T, DT * Wg[:, D:].T], axis=0)).astype(BF)

    ts = (DT * np.arange(NUM_STEPS)).astype(np.float32)
    b1r = b1.reshape(FK, P)                                        # [m, p]
    wtr = np.ascontiguousarray(W1[:, D]).reshape(FK, P)            # [m, p]
    b1e = b1r[None, :, :] + ts[:, None, None] * wtr[None, :, :]    # [s, m, p]
    b1e = np.ascontiguousarray(b1e.transpose(2, 0, 1).reshape(P, NUM_STEPS * FK))
    b2c = np.ascontiguousarray(b2.reshape(FK, P).T)
    bgc = np.ascontiguousarray(bg.reshape(FK, P).T)
    return dict(w1t=w1t, w2t=w2t, wgt=wgt,
                b1e=b1e.astype(np.float32),
                b2c=b2c.astype(np.float32), bgc=bgc.astype(np.float32))


def _host_prep_mir(W1, b1, W2, b2, Wg, bg):
    # Same fp8 W1/W2 scaling as z1, but the gate weights go to fp16
    # (fp16's 10 mantissa bits beat bf16's 7; range is tiny here).
    F8 = _fp8_np()
    W1 = np.asarray(W1, np.float32)
    W2 = np.asarray(W2, np.float32)
    Wg = np.asarray(Wg, np.float32)
    b1 = np.asarray(b1, np.float32)
    b2 = np.asarray(b2, np.float32)
    bg = np.asarray(bg, np.float32)

    w1t = np.ascontiguousarray((AW1 * DT * W1[:, :D]).T).astype(F8)
    w2t = np.ascontiguousarray((AW2 * W2).T).astype(F8)
    wgt = np.ascontiguousarray(
        np.concatenate([Wg[:, :D].T, DT * Wg[:, D:].T], axis=0)
    ).astype(np.float16)

    ts = (DT * np.arange(NUM_STEPS)).astype(np.float32)
    b1r = b1.reshape(FK, P)
    wtr = np.ascontiguousarray(W1[:, D]).reshape(FK, P)
    b1e = b1r[None, :, :] + ts[:, None, None] * wtr[None, :, :]
    b1e = np.ascontiguousarray(b1e.transpose(2, 0, 1).reshape(P, NUM_STEPS * FK))
    b2c = np.ascontiguousarray(b2.reshape(FK, P).T)
    bgc = np.ascontiguousarray(bg.reshape(FK, P).T)
    return dict(w1t=w1t, w2t=w2t, wgt=wgt,
                b1e=b1e.astype(np.float32),
                b2c=b2c.astype(np.float32), bgc=bgc.astype(np.float32))


def _make_in_map_mir(x_slice, h_slice, weights):
    F8 = _fp8_np()
    xTc = np.ascontiguousarray(x_slice.T)
    hTc = np.ascontiguousarray(h_slice.T) * np.float32(1.0 / DT)
    return dict(
        hT=hTc.astype(np.float16),
        h0b=hTc.astype(F8),
        xTb=xTc.astype(np.float16),
        **weights,
    )


def _make_in_map_z1(x_slice, h_slice, weights):
    F8 = _fp8_np()
    BF = ml_dtypes.bfloat16
    xTc = np.ascontiguousarray(x_slice.T)
    hTc = np.ascontiguousarray(h_slice.T) * np.float32(1.0 / DT)
    return dict(
        hT=hTc,
        h0b=hTc.astype(F8),
        xTb=xTc.astype(BF),
        **weights,
    )


def _make_in_map(x_slice, h_slice, weights, mode=MODE):
    cdt = _cast_dt(mode)
    xs = 1.0 if mode != "fp8" else AX
    xTc = np.ascontiguousarray(x_slice.T)
    return dict(
        hT=np.ascontiguousarray(h_slice.T) * np.float32(1.0 / DT),
        xT=xTc,
        xTb=(xTc * np.float32(xs)).astype(cdt) if mode == "fp8"
        else xTc.astype(cdt),
        **weights,
    )


def kernel(current_node_features, previous_hidden_state, W1, b1, W2, b2, Wg, bg):
    from concourse.bass_utils import run_bass_kernel_spmd

    x = np.asarray(current_node_features, np.float32)
    h0 = np.asarray(previous_hidden_state, np.float32)
    if MODE == "mir":
        weights = _host_prep_mir(W1, b1, W2, b2, Wg, bg)
        mk = _make_in_map_mir
    elif MODE == "z1":
        weights = _host_prep_z1(W1, b1, W2, b2, Wg, bg)
        mk = _make_in_map_z1
    else:
        weights = _host_prep(W1, b1, W2, b2, Wg, bg)
        mk = _make_in_map

    in_maps = []
    for c in range(NCORES):
        sl = slice(c * NPC, (c + 1) * NPC)
        in_maps.append(mk(x[sl], h0[sl], weights))

    nc = _get_nc(NPC)
    trace = bool(os.environ.get("BASS_TRACE"))
    if trace:
        try:
            import antenv.axon_hooks  # noqa: F401
        except ImportError:
            # no NTFF shim installed (see test.py) -> tracing would crash
            os.environ["BASS_NEVER_TRACE"] = "1"
            trace = False
    res = run_bass_kernel_spmd(nc, in_maps, core_ids=list(range(NCORES)),
                               trace=trace)
    LAST["res"] = res

    out = np.empty((N_TOTAL, D), np.float32)
    for c in range(NCORES):
        out[c * NPC:(c + 1) * NPC] = res.results[c]["outT"].T.astype(np.float32)
    return out, out



# revision 15
# speedup vs baseline: 1.0029x; 1.0029x over previous
"""Trainium2 Bass kernel for NeuralODEMemory (nn_NeuralODEMemory_28355374088720).

Math (reference):
    dt = 0.1, 10 Euler steps over h (N=65536 rows, D=512):
        z = [h, t]                              # time feature column
        deriv = tanh(tanh(z @ W1.T + b1) @ W2.T + b2)
        h <- h + dt * deriv
    gate  = sigmoid([x, h10] @ Wg.T + bg)
    out   = gate * h10 + (1 - gate) * x

Kernel strategy (MODE="mir", the default — ~770us; z1 fallback ~1017us):
  * See _build_mir's docstring for the winning mirror-fp8/fp16 design:
    jointly PE/ACT-bound, rb=2048 blocks, 2-slot PSUM rotation, fp16
    h'/d/gate (bf16 h' fails the 2e-2 gate: 2.5e-2; fp16 gives 1.02e-2),
    fp8 DR matmuls for the ODE, fp16 gate, half-block gate pieces popped
    into the next pair's step loop.

Older z1 strategy notes (MODE="z1"):
  * Data-parallel over 8 NeuronCores (8192 rows each); weights replicated.
  * Feature-major ("transposed") activation layout [D, rows] on chip so the
    per-step matmul chain needs no transposes: weights are the stationary
    operand ([in,out] chunks), activations stream as the moving operand, and
    each layer's PSUM output is already in the layout the next layer consumes.
  * The time-feature column is algebraically folded into a per-step bias:
    z @ W1.T = h @ W1[:, :D].T + t * W1[:, D], so b1_eff(s) = b1 + t_s*W1[:,D].
    Bias-add (and input scaling) is free via ACT: out = f(in*scale + bias).
  * h is kept "primed" as h' = h/dt with W1/WgB pre-scaled by dt on the host,
    so the per-step update is a single tensor_tensor add: h' += deriv.
  * z1-accumulation (matmul linearity): z1 = W1s @ h'_s is kept in fp32 SBUF
    and updated with z1 += W1s @ d_s, where d_s = layer-2's tanh output
    written by ACT directly in fp8 — the DoubleRow fp8 matmuls (full 2x MACs
    per instruction, LDWEIGHTS hidden) consume the previous step's ACT
    output with no elementwise op on the critical path.  The gate stays
    bf16 (fp8 gate operands alone cost 1.3-1.8e-2 rel err; see
    quantstudy.py).  Overall rel err ~1.1e-2 vs the 2e-2 gate.
  * Two row-blocks are software-pipelined with a phase-split issue order
    (A1 B1 A2 B2 per step) so the inter-engine dependency cycle
    (a-ACT -> L2 -> d-ACT -> L1delta -> z1-drain -> a-ACT) of one block is
    hidden under the other block's engine work.  GpSimd absorbs group B's
    h-adds; everything else elementwise is merged into large contiguous DVE
    ops.  Each pair's gate is split into per-chunk pieces issued one per
    step inside the NEXT pair's loop -- dependency-free filler the engines
    execute during their per-step waits (engines drain queues in order, so
    only ready work placed ahead of a stalled instruction can fill a gap).
  * MODE="bf16" (plain bf16 matmuls) and MODE="fp8" (the older h-mirror fp8
    design) are kept for reference/fallback via NODE_KERNEL_MODE.
  * Host does the cheap prep: weight transpose/scale/cast, x/h transposes.
"""

import os
from contextlib import ExitStack

import numpy as np
import ml_dtypes

N_TOTAL = 65536
D = 512
NCORES = 8
NPC = N_TOTAL // NCORES          # rows per core
NUM_STEPS = 10
TIME_INTERVAL = 1.0
DT = TIME_INTERVAL / NUM_STEPS
P = 128
FK = D // P                      # feature chunks of 128 (4)
MMN = 512                        # matmul moving-operand free dim (one PSUM bank)

MODE = os.environ.get("NODE_KERNEL_MODE", "mir")

# fp8 (e4m3) power-of-2 scale factors
AH = 4.0                         # h' mirror scale
AW1 = 2.0 ** 15                  # on dt*W1s  (|dt*W1s| <= 0.0044 -> <= 145)
AW2 = 2.0 ** 12                  # on W2      (|W2| <= 0.0442 -> <= 181)
AX = 32.0                        # x mirror   (|x| <= ~5.3 -> <= 170)
CG = 2.0 ** 16                   # common gate scale: awga*AX = awgb*AH = CG
AWGA = CG / AX                   # 2048  (|WgA| <= 0.0313 -> <= 64)
AWGB = CG / AH                   # 16384 (|dt*WgB| <= 0.0032 -> <= 52)

_CACHE = {}
LAST = {}                        # stash of the last run's BassKernelResults


def _block_schedule(npc, rblk):
    # Uniform blocks: lead blocks smaller than rblk were tried to shorten the
    # serial head, but 512-row blocks starve the PE mid-kernel (PSUM slots and
    # short matmul groups can't hide the ACT/DVE tails) — a net loss.
    return [(i * rblk, rblk) for i in range(npc // rblk)]


def _build(npc, rblk=1024, mode=MODE):
    import concourse.mybir as mybir
    import concourse.tile as tile
    from concourse import bacc

    f32 = mybir.dt.float32
    bf16 = mybir.dt.bfloat16
    fp8 = mybir.dt.float8e4
    Act = mybir.ActivationFunctionType
    Alu = mybir.AluOpType
    DR = mybir.MatmulPerfMode.DoubleRow

    # per-tensor matmul dtypes: "fp8" = everything, "fp8l2" = only layer 2
    # (a and W2 — both well-conditioned for e4m3), else bf16
    cdt = fp8 if mode == "fp8" else bf16          # h-mirror / x-mirror / W1 / Wg
    l2dt = fp8 if mode in ("fp8", "fp8l2") else bf16   # a / W2
    dr1 = mode == "fp8"                            # DoubleRow on layer 1 + gate
    dr2 = mode in ("fp8", "fp8l2")                 # DoubleRow on layer 2
    blocks = _block_schedule(npc, rblk)

    nc = bacc.Bacc("TRN2", target_bir_lowering=False, debug=False,
                   num_devices=NCORES)

    hT = nc.dram_tensor("hT", [D, npc], f32, kind="ExternalInput").ap()
    xT = nc.dram_tensor("xT", [D, npc], f32, kind="ExternalInput").ap()
    xTb = nc.dram_tensor("xTb", [D, npc], cdt, kind="ExternalInput").ap()
    w1t = nc.dram_tensor("w1t", [D, D], cdt, kind="ExternalInput").ap()
    w2t = nc.dram_tensor("w2t", [D, D], l2dt, kind="ExternalInput").ap()
    wgt = nc.dram_tensor("wgt", [2 * D, D], cdt, kind="ExternalInput").ap()
    b1e = nc.dram_tensor("b1e", [P, NUM_STEPS * FK], f32, kind="ExternalInput").ap()
    b2c = nc.dram_tensor("b2c", [P, FK], f32, kind="ExternalInput").ap()
    bgc = nc.dram_tensor("bgc", [P, FK], f32, kind="ExternalInput").ap()
    outT = nc.dram_tensor("outT", [D, npc], fp16, kind="ExternalOutput").ap()

    hTr = hT.rearrange("(k p) r -> p k r", p=P)
    xTr = xT.rearrange("(k p) r -> p k r", p=P)
    xTbr = xTb.rearrange("(k p) r -> p k r", p=P)
    outTr = outT.rearrange("(k p) r -> p k r", p=P)

    # ACT input scales that undo the fp8 operand scaling exactly
    s_l1 = 1.0 / (AW1 * AH) if dr1 else 1.0
    s_l2 = 1.0 / AW2 if dr2 else 1.0
    s_g = 1.0 / CG if dr1 else 1.0

    with tile.TileContext(nc) as tc:
        with ExitStack() as ctx:
            consts = ctx.enter_context(tc.tile_pool(name="consts", bufs=1))
            hp = ctx.enter_context(tc.tile_pool(name="h", bufs=4))
            hbp = ctx.enter_context(tc.tile_pool(name="hb", bufs=2))
            apool = ctx.enter_context(tc.tile_pool(name="a", bufs=2))
            dpool = ctx.enter_context(tc.tile_pool(name="d", bufs=2))
            xbp = ctx.enter_context(tc.tile_pool(name="xb", bufs=4))
            xfp = ctx.enter_context(tc.tile_pool(name="xf", bufs=2))
            gp = ctx.enter_context(tc.tile_pool(name="g", bufs=2))
            scp = ctx.enter_context(tc.tile_pool(name="sc", bufs=6))
            psp = ctx.enter_context(tc.tile_pool(name="ps", bufs=4, space="PSUM"))

            # Replicated constants, resident for the whole kernel.
            w1 = consts.tile([P, FK, D], cdt)
            nc.sync.dma_start(w1[:], w1t.rearrange("(k p) m -> p k m", p=P))
            w2 = consts.tile([P, FK, D], l2dt)
            nc.sync.dma_start(w2[:], w2t.rearrange("(k p) m -> p k m", p=P))
            wg = consts.tile([P, 2 * FK, D], cdt)
            nc.sync.dma_start(wg[:], wgt.rearrange("(k p) m -> p k m", p=P))
            b1 = consts.tile([P, NUM_STEPS * FK], f32)
            nc.sync.dma_start(b1[:], b1e)
            b2 = consts.tile([P, FK], f32)
            nc.sync.dma_start(b2[:], b2c)
            bg = consts.tile([P, FK], f32)
            nc.sync.dma_start(bg[:], bgc)

            def mm_group(ps_tile, wtile, wk, m, rhs_tile, rk, start, stop,
                         nsub, dr):
                for sub in range(nsub):
                    if dr:
                        # DoubleRow: one matmul contracts a pair of k-chunks
                        nc.tensor.matmul(
                            ps_tile[:, sub * MMN:(sub + 1) * MMN],
                            wtile[:, 2 * wk:2 * wk + 2, m * P:(m + 1) * P],
                            rhs_tile[:, 2 * rk:2 * rk + 2,
                                     sub * MMN:(sub + 1) * MMN],
                            start=start, stop=stop, perf_mode=DR,
                        )
                    else:
                        nc.tensor.matmul(
                            ps_tile[:, sub * MMN:(sub + 1) * MMN],
                            wtile[:, wk, m * P:(m + 1) * P],
                            rhs_tile[:, rk, sub * MMN:(sub + 1) * MMN],
                            start=start, stop=stop,
                        )

            NK1 = FK // 2 if dr1 else FK      # layer-1 / gate k-group count
            NK2 = FK // 2 if dr2 else FK      # layer-2 k-group count

            def mirror(dst, src):
                # bf16/fp8 matmul mirror of the fp32 h' accumulator
                if mode == "fp8":
                    nc.vector.tensor_scalar_mul(dst, src, AH)
                else:
                    nc.vector.tensor_copy(dst, src)

            for rs, rblk_b in blocks:
                nsub = rblk_b // MMN
                h = hp.tile([P, FK, rblk_b], f32, tag="h")
                # per-chunk loads so each mirror cast starts as soon as its
                # chunk lands (matters for the first block's serial head)
                for k in range(FK):
                    nc.sync.dma_start(h[:, k], hTr[:, k, rs:rs + rblk_b])
                xb = xbp.tile([P, FK, rblk_b], cdt, tag="xb")
                nc.sync.dma_start(xb[:], xTbr[:, :, rs:rs + rblk_b])
                xf = xfp.tile([P, FK, rblk_b], f32, tag="xf")
                nc.sync.dma_start(xf[:], xTr[:, :, rs:rs + rblk_b])

                hb = hbp.tile([P, FK, rblk_b], cdt, tag="hb")
                for k in range(FK):
                    mirror(hb[:, k], h[:, k])

                for s in range(NUM_STEPS):
                    # layer 1: a = tanh((dt*W1s).T-chunks @ h'b + b1_eff(s))
                    a = apool.tile([P, FK, rblk_b], l2dt, tag="a")
                    for m in range(FK):
                        ps = psp.tile([P, rblk_b], f32, tag="ps")
                        for k in range(NK1):
                            mm_group(ps, w1, k, m, hb, k, k == 0, k == NK1 - 1,
                                     nsub, dr1)
                        col = s * FK + m
                        nc.scalar.activation(a[:, m], ps[:], Act.Tanh,
                                             bias=b1[:, col:col + 1], scale=s_l1)
                    # layer 2: deriv = tanh(W2.T-chunks @ a + b2)
                    d = dpool.tile([P, FK, rblk_b], bf16, tag="d")
                    for m in range(FK):
                        ps = psp.tile([P, rblk_b], f32, tag="ps")
                        for k in range(NK2):
                            mm_group(ps, w2, k, m, a, k, k == 0, k == NK2 - 1,
                                     nsub, dr2)
                        nc.scalar.activation(d[:, m], ps[:], Act.Tanh,
                                             bias=b2[:, m:m + 1], scale=s_l2)
                    # h' += deriv ; refresh matmul mirror.  Split the fp32
                    # adds across DVE and GpSimd to balance engine load.
                    hb = hbp.tile([P, FK, rblk_b], cdt, tag="hb")
                    for k in range(FK):
                        eng = nc.vector if (mode != "fp8" or k < 2) else nc.gpsimd
                        eng.tensor_tensor(h[:, k], h[:, k], d[:, k], Alu.add)
                        mirror(hb[:, k], h[:, k])

                # gate + combine, per output feature chunk
                for m in range(FK):
                    ps = psp.tile([P, rblk_b], f32, tag="ps")
                    for k in range(NK1):
                        mm_group(ps, wg, k, m, xb, k, k == 0, False, nsub, dr1)
                    for k in range(NK1):
                        mm_group(ps, wg, NK1 + k, m, hb, k, False,
                                 k == NK1 - 1, nsub, dr1)
                    g = gp.tile([P, rblk_b], f32, tag="g")
                    nc.scalar.activation(g[:], ps[:], Act.Sigmoid,
                                         bias=bg[:, m:m + 1], scale=s_g)
                    # out = x + g * (dt*h' - x)
                    dif = scp.tile([P, rblk_b], f32, tag="sc")
                    nc.vector.scalar_tensor_tensor(
                        dif[:], h[:, m], float(DT), xf[:, m],
                        Alu.mult, Alu.subtract)
                    gd = scp.tile([P, rblk_b], f32, tag="sc")
                    nc.vector.tensor_tensor(gd[:], g[:], dif[:], Alu.mult)
                    ot = scp.tile([P, rblk_b], f32, tag="sc")
                    nc.vector.tensor_tensor(ot[:], xf[:, m], gd[:], Alu.add)
                    nc.sync.dma_start(outTr[:, m, rs:rs + rblk_b], ot[:])

    nc.compile()
    return nc


def _build_z1(npc, rblk=1024):
    """z1-accumulation architecture, fp8 ODE loop + bf16 gate (v4).

    Math: keep z1 = (AW1*dt*W1) @ h'_s in fp32 SBUF, update z1 += W1s @ d_s
    where d_s is layer-2's fp8 ACT output (matmul linearity) -- no per-step
    h-mirror.  h' (fp32) accumulates d_s off the critical path.

    Scheduling: the per-step dependency cycle (a-ACT -> L2 MMs -> d-ACT ->
    L1delta MMs -> z1 drain -> next a-ACT) is ~2x longer than any engine's
    per-step work, so TWO adjacent row blocks are SOFTWARE-PIPELINED: all
    engines fill group A's chain gaps with group B's work.  With that slack,
    GpSimd (slow but otherwise idle) safely absorbs group B's h-adds and the
    gate's gd/ot products.  DVE ops are merged into big contiguous APs
    ([P, 2, rblk] psum pair drains, [P, FK*rblk] h-adds).
    Accuracy (CPU sim ~1.14e-2, HW ~1.09e-2): fp8 bounds from quantstudy.py
    -- gate must stay bf16, h' must stay fp32.
    """
    import concourse.mybir as mybir
    import concourse.tile as tile
    from concourse import bacc

    f32 = mybir.dt.float32
    bf16 = mybir.dt.bfloat16
    fp8 = mybir.dt.float8e4
    Act = mybir.ActivationFunctionType
    Alu = mybir.AluOpType
    DR = mybir.MatmulPerfMode.DoubleRow

    blocks = _block_schedule(npc, rblk)
    pairs = [(blocks[i], blocks[i + 1]) for i in range(0, len(blocks), 2)]

    nc = bacc.Bacc("TRN2", target_bir_lowering=False, debug=False,
                   num_devices=NCORES)

    hT = nc.dram_tensor("hT", [D, npc], f32, kind="ExternalInput").ap()
    h0b = nc.dram_tensor("h0b", [D, npc], fp8, kind="ExternalInput").ap()
    xTb = nc.dram_tensor("xTb", [D, npc], bf16, kind="ExternalInput").ap()
    w1t = nc.dram_tensor("w1t", [D, D], fp8, kind="ExternalInput").ap()
    w2t = nc.dram_tensor("w2t", [D, D], fp8, kind="ExternalInput").ap()
    wgt = nc.dram_tensor("wgt", [2 * D, D], bf16, kind="ExternalInput").ap()
    b1e = nc.dram_tensor("b1e", [P, NUM_STEPS * FK], f32, kind="ExternalInput").ap()
    b2c = nc.dram_tensor("b2c", [P, FK], f32, kind="ExternalInput").ap()
    bgc = nc.dram_tensor("bgc", [P, FK], f32, kind="ExternalInput").ap()
    outT = nc.dram_tensor("outT", [D, npc], f32, kind="ExternalOutput").ap()

    hTr = hT.rearrange("(k p) r -> p k r", p=P)
    h0br = h0b.rearrange("(k p) r -> p k r", p=P)
    xTbr = xTb.rearrange("(k p) r -> p k r", p=P)
    outTr = outT.rearrange("(k p) r -> p k r", p=P)

    s_l1 = 1.0 / AW1
    s_l2 = 1.0 / AW2
    NKG = FK // 2                 # DR k-group count (2)

    with tile.TileContext(nc) as tc:
        with ExitStack() as ctx:
            consts = ctx.enter_context(tc.tile_pool(name="consts", bufs=1))
            z1p = ctx.enter_context(tc.tile_pool(name="z1", bufs=2))
            hp = ctx.enter_context(tc.tile_pool(name="h", bufs=4))
            hb0p = ctx.enter_context(tc.tile_pool(name="hb0", bufs=1))
            apool = ctx.enter_context(tc.tile_pool(name="a", bufs=2))
            dpool = ctx.enter_context(tc.tile_pool(name="d", bufs=4))
            xbp = ctx.enter_context(tc.tile_pool(name="xb", bufs=4))
            hgp = ctx.enter_context(tc.tile_pool(name="hg", bufs=2))
            gp = ctx.enter_context(tc.tile_pool(name="g", bufs=1))
            scp = ctx.enter_context(tc.tile_pool(name="sc", bufs=4))
            psp = ctx.enter_context(tc.tile_pool(name="ps", bufs=4, space="PSUM"))

            # DMA order matters for the serial head: the first L1 fill
            # needs only w1 + hb0(block A) + b1, so those go first; the
            # gate-only constants (wg, bg) queue last.
            w1 = consts.tile([P, FK, D], fp8)
            nc.sync.dma_start(w1[:], w1t.rearrange("(k p) m -> p k m", p=P))
            b1 = consts.tile([P, NUM_STEPS * FK], f32)
            nc.sync.dma_start(b1[:], b1e)
            w2 = consts.tile([P, FK, D], fp8)
            nc.sync.dma_start(w2[:], w2t.rearrange("(k p) m -> p k m", p=P))
            b2 = consts.tile([P, FK], f32)
            nc.sync.dma_start(b2[:], b2c)
            wg = consts.tile([P, 2 * FK, D], fp16)
            nc.sync.dma_start(wg[:], wgt.rearrange("(k p) m -> p k m", p=P))
            bg = consts.tile([P, FK], f32)
            nc.sync.dma_start(bg[:], bgc)

            def dr_one(ps, wtile, m, rhs, nsub):
                # fill psum tile ps[P, rblk] with output chunk m
                for kg in range(NKG):
                    for sub in range(nsub):
                        nc.tensor.matmul(
                            ps[:, sub * MMN:(sub + 1) * MMN],
                            wtile[:, 2 * kg:2 * kg + 2, m * P:(m + 1) * P],
                            rhs[:, 2 * kg:2 * kg + 2,
                                sub * MMN:(sub + 1) * MMN],
                            start=kg == 0, stop=kg == NKG - 1, perf_mode=DR,
                        )

            def load_group(rs, rb, gi):
                h = hp.tile([P, FK, rb], f32, tag="h", name=f"h{gi}")
                for k in range(FK):
                    nc.sync.dma_start(h[:, k], hTr[:, k, rs:rs + rb])
                hb0 = hb0p.tile([P, FK, rb], fp8, tag="hb0", name=f"hb0{gi}")
                nc.sync.dma_start(hb0[:], h0br[:, :, rs:rs + rb])
                xb = xbp.tile([P, FK, rb], bf16, tag="xb", name=f"xb{gi}")
                nc.sync.dma_start(xb[:], xTbr[:, :, rs:rs + rb])
                return h, hb0, xb

            def init_group(hb0, rb, gi):
                nsub = rb // MMN
                z1 = z1p.tile([P, FK, rb], f32, tag="z1", name=f"z1{gi}")
                for m in range(FK):
                    ps = psp.tile([P, rb], f32, tag="ps", name=f"psi{gi}")
                    dr_one(ps, w1, m, hb0, nsub)
                    nc.vector.tensor_copy(z1[:, m], ps[:])
                return z1

            def step_part1(s, z1, rb, gi):
                # a = tanh(z1/AW1 + b1_eff(s)) then d = tanh(W2 @ a/AW2 + b2)
                nsub = rb // MMN
                a = apool.tile([P, FK, rb], fp8, tag="a", name=f"a{gi}")
                for m in range(FK):
                    col = s * FK + m
                    nc.scalar.activation(a[:, m], z1[:, m], Act.Tanh,
                                         bias=b1[:, col:col + 1], scale=s_l1)
                d = dpool.tile([P, FK, rb], fp8, tag="d", name=f"d{gi}")
                for m in range(FK):
                    ps = psp.tile([P, rb], f32, tag="ps", name=f"ps2{gi}")
                    dr_one(ps, w2, m, a, nsub)
                    nc.scalar.activation(d[:, m], ps[:], Act.Tanh,
                                         bias=b2[:, m:m + 1], scale=s_l2)
                return d

            def step_part2(s, z1, h, d, rb, gi):
                # z1 += W1s @ d first (gates next step's a-ACT), then h' += d
                # (merged; group B rides GpSimd except the last step, whose
                # h-add gates the gate phase)
                nsub = rb // MMN
                if s < NUM_STEPS - 1:
                    for m in range(FK):
                        ps = psp.tile([P, rb], f32, tag="ps", name=f"ps1{gi}")
                        dr_one(ps, w1, m, d, nsub)
                        nc.vector.tensor_tensor(z1[:, m], z1[:, m],
                                                ps[:], Alu.add)
                heng = nc.gpsimd if (gi == 1 and s < NUM_STEPS - 1) else nc.vector
                heng.tensor_tensor(h[:], h[:], d[:], Alu.add)

            def gate_prep(h, gi):
                # bf16 mirror of h' for the gate matmul (after the last h-add)
                rb = h.shape[2]
                hbg = hgp.tile([P, FK, rb], bf16, tag="hbg", name=f"hbg{gi}")
                nc.vector.tensor_copy(hbg[:], h[:])
                return hbg

            def gate_piece(h, xb, hbg, rs, rb, gi, m):
                # one output chunk of the gate: issued as ready filler work
                # inside the NEXT pair's step loop
                nsub = rb // MMN
                if True:
                    ps = psp.tile([P, rb], f32, tag="ps", name=f"psg{gi}")
                    for sub in range(nsub):
                        sl = slice(sub * MMN, (sub + 1) * MMN)
                        for k in range(FK):
                            nc.tensor.matmul(
                                ps[:, sl], wg[:, k, m * P:(m + 1) * P],
                                xb[:, k, sl], start=k == 0, stop=False)
                        for k in range(FK):
                            nc.tensor.matmul(
                                ps[:, sl],
                                wg[:, FK + k, m * P:(m + 1) * P],
                                hbg[:, k, sl], start=False,
                                stop=k == FK - 1)
                    if True:
                        g = gp.tile([P, rb], f32, tag="g", name=f"g{gi}")
                        nc.scalar.activation(g[:], ps[:], Act.Sigmoid,
                                             bias=bg[:, m:m + 1], scale=1.0)
                        # out = x + g*(dt*h' - x), x read as bf16 (xb)
                        dif = scp.tile([P, rb], f32, tag="sc", name=f"dif{gi}")
                        nc.vector.scalar_tensor_tensor(
                            dif[:], h[:, m], float(DT), xb[:, m],
                            Alu.mult, Alu.subtract)
                        gd = scp.tile([P, rb], f32, tag="sc", name=f"gd{gi}")
                        nc.vector.tensor_tensor(gd[:], g[:], dif[:], Alu.mult)
                        ot = scp.tile([P, rb], f32, tag="sc", name=f"ot{gi}")
                        nc.vector.tensor_tensor(ot[:], xb[:, m], gd[:], Alu.add)
                        nc.sync.dma_start(outTr[:, m, rs:rs + rb], ot[:])

            pend = []
            for (rsA, rbA), (rsB, rbB) in pairs:
                hA, hb0A, xbA = load_group(rsA, rbA, 0)
                hB, hb0B, xbB = load_group(rsB, rbB, 1)
                z1A = init_group(hb0A, rbA, 0)
                z1B = init_group(hb0B, rbB, 1)
                for s in range(NUM_STEPS):
                    dA = step_part1(s, z1A, rbA, 0)
                    dB = step_part1(s, z1B, rbB, 1)
                    step_part2(s, z1A, hA, dA, rbA, 0)
                    step_part2(s, z1B, hB, dB, rbB, 1)
                    if pend:
                        pend.pop(0)()
                # previous pair's gate pieces not yet drained (first 2 steps
                # of this pair had none queued yet on the very first pair)
                while pend:
                    pend.pop(0)()
                hbgA = gate_prep(hA, 0)
                hbgB = gate_prep(hB, 1)
                pend = [
                    (lambda h=h_, xb=xb_, hbg=hbg_, rs=rs_, rb=rb_, gi=gi_,
                            m=m_: gate_piece(h, xb, hbg, rs, rb, gi, m))
                    for (h_, xb_, hbg_, rs_, rb_, gi_) in
                    ((hA, xbA, hbgA, rsA, rbA, 0), (hB, xbB, hbgB, rsB, rbB, 1))
                    for m_ in range(FK)
                ]
            while pend:
                pend.pop(0)()

    nc.compile()
    return nc


def _build_mir(npc, rb=2048):
    """Mirror-fp8 architecture (v5) — no z1 accumulator, ACT-bound pipeline.

    Per step (per 2048-row block):
        L1 psum = (AW1*dt*W1).T @ hb      fp8 DR matmuls, 8 insts per m
        a  = tanh(psum/AW1 + b1e(s,m))    ACT psum->SBUF fp8, FD=2048
        L2 psum = (AW2*W2).T @ a          fp8 DR
        d  = tanh(psum/AW2 + b2)          ACT psum->SBUF bf16
        h += d                            DVE bf16 TT (2x mode), per k-chunk
        hb = fp8(h)                       DVE copy-cast, per k-pair
    h' = h/dt is kept in bf16 (quantstudy2: adds ~1e-3 vs fp32; total rel
    err ~1.0-1.2e-2 incl. fp8 W1/W2/a/hb quantization).  The gate matmul
    reads h (bf16) directly — no separate bf16 gate mirror.

    Why this beats z1 (1017us): microbench (bench1.py) showed matmuls
    sustain 216ns (LDWEIGHTS fully hidden even with changing stationary)
    and ACT calls carry a fixed ~360ns overhead, so the kernel is jointly
    PE/ACT-bound at ~676/683us with FD=2048 ACT drains.  z1's fp32 psum
    drains (390us DVE) are replaced by cheap bf16 adds + fp8 casts, and
    rb=2048 halves the ACT call count vs rb=1024.

    PSUM: 2 slots of [P, 2048] f32 (4 banks each) ping-ponged through one
    pool tag; ACT drains a whole m-chunk per call.  Two blocks run their
    10 steps together (pair); the previous pair's gate pieces + xb loads
    pop one-per-step inside the next pair's loop as PE/ACT filler.
    """
    import concourse.mybir as mybir
    import concourse.tile as tile
    from concourse import bacc

    f32 = mybir.dt.float32
    fp16 = mybir.dt.float16
    fp8 = mybir.dt.float8e4
    Act = mybir.ActivationFunctionType
    Alu = mybir.AluOpType
    DR = mybir.MatmulPerfMode.DoubleRow

    nsub = rb // MMN
    nblk = npc // rb
    assert nblk % 2 == 0
    pairs = [(2 * i, 2 * i + 1) for i in range(nblk // 2)]

    nc = bacc.Bacc("TRN2", target_bir_lowering=False, debug=False,
                   num_devices=NCORES)

    hT = nc.dram_tensor("hT", [D, npc], fp16, kind="ExternalInput").ap()
    h0b = nc.dram_tensor("h0b", [D, npc], fp8, kind="ExternalInput").ap()
    xTb = nc.dram_tensor("xTb", [D, npc], fp16, kind="ExternalInput").ap()
    w1t = nc.dram_tensor("w1t", [D, D], fp8, kind="ExternalInput").ap()
    w2t = nc.dram_tensor("w2t", [D, D], fp8, kind="ExternalInput").ap()
    wgt = nc.dram_tensor("wgt", [2 * D, D], fp16, kind="ExternalInput").ap()
    b1e = nc.dram_tensor("b1e", [P, NUM_STEPS * FK], f32, kind="ExternalInput").ap()
    b2c = nc.dram_tensor("b2c", [P, FK], f32, kind="ExternalInput").ap()
    bgc = nc.dram_tensor("bgc", [P, FK], f32, kind="ExternalInput").ap()
    outT = nc.dram_tensor("outT", [D, npc], f32, kind="ExternalOutput").ap()

    hTr = hT.rearrange("(k p) r -> p k r", p=P)
    h0br = h0b.rearrange("(k p) r -> p k r", p=P)
    xTbr = xTb.rearrange("(k p) r -> p k r", p=P)
    outTr = outT.rearrange("(k p) r -> p k r", p=P)

    s_l1 = 1.0 / AW1
    s_l2 = 1.0 / AW2

    with tile.TileContext(nc) as tc:
        with ExitStack() as ctx:
            consts = ctx.enter_context(tc.tile_pool(name="consts", bufs=1))
            hp = ctx.enter_context(tc.tile_pool(name="h", bufs=4))
            hbp = ctx.enter_context(tc.tile_pool(name="hb", bufs=3))
            apool = ctx.enter_context(tc.tile_pool(name="a", bufs=2))
            dpool = ctx.enter_context(tc.tile_pool(name="d", bufs=2))
            xbp = ctx.enter_context(tc.tile_pool(name="xb", bufs=2))
            gp = ctx.enter_context(tc.tile_pool(name="g", bufs=1))
            scp = ctx.enter_context(tc.tile_pool(name="sc", bufs=2))
            otp = ctx.enter_context(tc.tile_pool(name="ot", bufs=2))
            psp = ctx.enter_context(tc.tile_pool(name="ps", bufs=2, space="PSUM"))

            w1 = consts.tile([P, FK, D], fp8)
            nc.sync.dma_start(w1[:], w1t.rearrange("(k p) m -> p k m", p=P))
            w2 = consts.tile([P, FK, D], fp8)
            nc.sync.dma_start(w2[:], w2t.rearrange("(k p) m -> p k m", p=P))
            wg = consts.tile([P, 2 * FK, D], bf16)
            nc.sync.dma_start(wg[:], wgt.rearrange("(k p) m -> p k m", p=P))
            b1 = consts.tile([P, NUM_STEPS * FK], f32)
            nc.sync.dma_start(b1[:], b1e)
            b2 = consts.tile([P, FK], f32)
            nc.sync.dma_start(b2[:], b2c)
            bg = consts.tile([P, FK], f32)
            nc.sync.dma_start(bg[:], bgc)

            # First ACT instruction is a sigmoid: loads the sigmoid table
            # set (which also contains tanh) once, during the DMA head,
            # instead of a tanh-set load now plus a sigmoid-set switch
            # stalling the ACT stream at the first gate piece.
            warm = consts.tile([P, 1], f32)
            nc.scalar.activation(warm[:], b2[:, 0:1],
                                 Act.Sigmoid, bias=0.0, scale=1.0)

            def dr_fill(ps, wtile, m, rhs):
                # one m-chunk of a 512-contract fp8 DR matmul group
                for kg in range(FK // 2):
                    for sub in range(nsub):
                        nc.tensor.matmul(
                            ps[:, sub * MMN:(sub + 1) * MMN],
                            wtile[:, 2 * kg:2 * kg + 2, m * P:(m + 1) * P],
                            rhs[:, 2 * kg:2 * kg + 2,
                                sub * MMN:(sub + 1) * MMN],
                            start=kg == 0, stop=kg == FK // 2 - 1,
                            perf_mode=DR,
                        )

            def load_block(rs, b):
                hb = hbp.tile([P, FK, rb], fp8, tag="hb", name=f"hb{b}")
                nc.sync.dma_start(hb[:, 0:2], h0br[:, 0:2, rs:rs + rb])
                nc.sync.dma_start(hb[:, 2:4], h0br[:, 2:4, rs:rs + rb])
                h = hp.tile([P, FK, rb], fp16, tag="h", name=f"h{b}")
                for k in range(FK):
                    nc.sync.dma_start(h[:, k], hTr[:, k, rs:rs + rb])
                return h, hb

            def l1_phase(s, b, hb):
                a = apool.tile([P, FK, rb], fp8, tag="a", name=f"a{b}")
                for m in range(FK):
                    ps = psp.tile([P, rb], f32, tag="ps", name=f"ps1{b}")
                    dr_fill(ps, w1, m, hb)
                    col = s * FK + m
                    nc.scalar.activation(a[:, m], ps[:], Act.Tanh,
                                         bias=b1[:, col:col + 1], scale=s_l1)
                return a

            def l2_phase(s, b, a, h):
                # L2 matmuls + d-ACT, then h += d per chunk and the fp8
                # re-mirror per chunk-pair (skipped on the last step)
                d = dpool.tile([P, FK, rb], fp16, tag="d", name=f"d{b}")
                hbn = None
                if s < NUM_STEPS - 1:
                    hbn = hbp.tile([P, FK, rb], fp8, tag="hb", name=f"hbn{b}")
                for m in range(FK):
                    ps = psp.tile([P, rb], f32, tag="ps", name=f"ps2{b}")
                    dr_fill(ps, w2, m, a)
                    nc.scalar.activation(d[:, m], ps[:], Act.Tanh,
                                         bias=b2[:, m:m + 1], scale=s_l2)
                    nc.vector.tensor_tensor(h[:, m], h[:, m], d[:, m], Alu.add)
                    if hbn is not None and m % 2 == 1:
                        kg = m // 2
                        nc.vector.tensor_copy(hbn[:, 2 * kg:2 * kg + 2],
                                              h[:, 2 * kg:2 * kg + 2])
                return hbn

            def load_xb(rs, b):
                xb = xbp.tile([P, FK, rb], fp16, tag="xb", name=f"xb{b}")
                nc.sync.dma_start(xb[:], xTbr[:, :, rs:rs + rb])
                return xb

            def gate_piece(h, xb, rs, b, m, ro, w):
                # Gate piece over rows [ro, ro+w) of the block.  ODE pops use
                # w=512 (an 8-inst fill shorter than one a/d-ACT drain, so the
                # 2-slot psum rotation never starves ACT); the tail, which is
                # PE-bound, uses w=1024 for denser fills.
                ps = psp.tile([P, w], f32, tag="ps", name=f"psg{b}")
                for k in range(2 * FK):
                    rhs = xb[:, k] if k < FK else h[:, k - FK]
                    for sub in range(w // MMN):
                        sl = slice(sub * MMN, (sub + 1) * MMN)
                        so = slice(ro + sub * MMN, ro + (sub + 1) * MMN)
                        nc.tensor.matmul(ps[:, sl],
                                         wg[:, k, m * P:(m + 1) * P],
                                         rhs[:, so], start=k == 0,
                                         stop=k == 2 * FK - 1)
                g = gp.tile([P, w], fp16, tag="g", name=f"g{b}")
                nc.scalar.activation(g[:], ps[:], Act.Sigmoid,
                                     bias=bg[:, m:m + 1], scale=1.0)
                # out = xb + g * (dt*h' - xb), fp16 intermediates
                dif = scp.tile([P, w], fp16, tag="sc", name=f"dif{b}")
                nc.vector.scalar_tensor_tensor(dif[:], h[:, m, ro:ro + w],
                                               float(DT), xb[:, m, ro:ro + w],
                                               Alu.mult, Alu.subtract)
                gd = scp.tile([P, w], fp16, tag="sc", name=f"gd{b}")
                nc.vector.tensor_tensor(gd[:], g[:], dif[:], Alu.mult)
                ot = otp.tile([P, w], fp16, tag="ot", name=f"ot{b}")
                nc.vector.tensor_tensor(ot[:], xb[:, m, ro:ro + w],
                                        gd[:], Alu.add)
                nc.sync.dma_start(outTr[:, m, rs + ro:rs + ro + w], ot[:])

            pend = []
            for pi, (bA, bB) in enumerate(pairs):
                last = pi == len(pairs) - 1
                rsA, rsB = bA * rb, bB * rb
                hA, hbA = load_block(rsA, bA)
                hB, hbB = load_block(rsB, bB)
                for s in range(NUM_STEPS):
                    aA = l1_phase(s, bA, hbA)
                    aB = l1_phase(s, bB, hbB)
                    if pend:
                        pend.pop(0)[1]()
                    hbA2 = l2_phase(s, bA, aA, hA)
                    hbB2 = l2_phase(s, bB, aB, hB)
                    hbA, hbB = hbA2, hbB2
                    if pend:
                        pend.pop(0)[1]()
                while pend:
                    pend.pop(0)[1]()
                for h_, rs_, b_ in ((hA, rsA, bA), (hB, rsB, bB)):
                    def mk_load(h=h_, rs=rs_, b=b_):
                        state = {}

                        def go():
                            state["xb"] = load_xb(rs, b)
                        return state, go
                    state, go = mk_load()
                    pend.append(("load", go))
                    w_ = 1024
                    for m_ in range(FK):
                        for q_ in range(rb // w_):
                            pend.append(
                                ("piece",
                                 lambda h=h_, rs=rs_, b=b_, m=m_,
                                        ro=q_ * w_, w=w_, st=state:
                                 gate_piece(h, st["xb"], rs, b, m, ro, w)))
                if last:
                    # tail: issue the xb DMAs first (xbp is double-buffered)
                    # so the gate pieces never wait on a load
                    pend.sort(key=lambda e: e[0] != "load")
            while pend:
                pend.pop(0)[1]()

    nc.compile()
    return nc


def _get_nc(npc, rblk=1024, mode=MODE):
    key = (npc, rblk, mode)
    if key not in _CACHE:
        if mode == "mir":
            _CACHE[key] = _build_mir(npc)
        elif mode == "z1":
            _CACHE[key] = _build_z1(npc, rblk)
        else:
            _CACHE[key] = _build(npc, rblk, mode)
    return _CACHE[key]


def _fp8_np():
    import concourse.mybir as mybir
    return mybir.dt.np(mybir.dt.float8e4)


def _cast_dt(mode):
    # dtype of the x-mirror / W1 / Wg operands
    return _fp8_np() if mode == "fp8" else ml_dtypes.bfloat16


def _host_prep(W1, b1, W2, b2, Wg, bg, mode=MODE):
    cdt = _cast_dt(mode)
    W1 = np.asarray(W1, np.float32)
    W2 = np.asarray(W2, np.float32)
    Wg = np.asarray(Wg, np.float32)
    b1 = np.asarray(b1, np.float32)
    b2 = np.asarray(b2, np.float32)
    bg = np.asarray(bg, np.float32)

    if mode == "fp8":
        sw1, sw2, swga, swgb = AW1, AW2, AWGA, AWGB
    elif mode == "fp8l2":
        sw1, sw2, swga, swgb = 1.0, AW2, 1.0, 1.0
    else:
        sw1 = sw2 = swga = swgb = 1.0
    l2dt = _fp8_np() if mode in ("fp8", "fp8l2") else cdt

    w1t = np.ascontiguousarray((sw1 * DT * W1[:, :D]).T).astype(cdt)  # [in, out]
    w2t = np.ascontiguousarray((sw2 * W2).T).astype(l2dt)
    wgt = np.ascontiguousarray(
        np.concatenate([swga * Wg[:, :D].T, swgb * DT * Wg[:, D:].T],
                       axis=0)).astype(cdt)

    ts = (DT * np.arange(NUM_STEPS)).astype(np.float32)
    b1r = b1.reshape(FK, P)                                        # [m, p]
    wtr = np.ascontiguousarray(W1[:, D]).reshape(FK, P)            # [m, p]
    b1e = b1r[None, :, :] + ts[:, None, None] * wtr[None, :, :]    # [s, m, p]
    b1e = np.ascontiguousarray(b1e.transpose(2, 0, 1).reshape(P, NUM_STEPS * FK))
    b2c = np.ascontiguousarray(b2.reshape(FK, P).T)
    bgc = np.ascontiguousarray(bg.reshape(FK, P).T)
    return dict(w1t=w1t, w2t=w2t, wgt=wgt,
                b1e=b1e.astype(np.float32),
                b2c=b2c.astype(np.float32), bgc=bgc.astype(np.float32))


def _host_prep_z1(W1, b1, W2, b2, Wg, bg):
    F8 = _fp8_np()
    BF = ml_dtypes.bfloat16
    W1 = np.asarray(W1, np.float32)
    W2 = np.asarray(W2, np.float32)
    Wg = np.asarray(Wg, np.float32)
    b1 = np.asarray(b1, np.float32)
    b2 = np.asarray(b2, np.float32)
    bg = np.asarray(bg, np.float32)

    w1t = np.ascontiguousarray((AW1 * DT * W1[:, :D]).T).astype(F8)  # [in, out]
    w2t = np.ascontiguousarray((AW2 * W2).T).astype(F8)
    wgt = np.ascontiguousarray(
        np.concatenate([Wg[:, :D].T, DT * Wg[:, D:].T], axis=0)).astype(BF)

    ts = (DT * np.arange(NUM_STEPS)).astype(np.float32)
    b1r = b1.reshape(FK, P)                                        # [m, p]
    wtr = np.ascontiguousarray(W1[:, D]).reshape(FK, P)            # [m, p]
    b1e = b1r[None, :, :] + ts[:, None, None] * wtr[None, :, :]    # [s, m, p]
    b1e = np.ascontiguousarray(b1e.transpose(2, 0, 1).reshape(P, NUM_STEPS * FK))
    b2c = np.ascontiguousarray(b2.reshape(FK, P).T)
    bgc = np.ascontiguousarray(bg.reshape(FK, P).T)
    return dict(w1t=w1t, w2t=w2t, wgt=wgt,
                b1e=b1e.astype(np.float32),
                b2c=b2c.astype(np.float32), bgc=bgc.astype(np.float32))


def _host_prep_mir(W1, b1, W2, b2, Wg, bg):
    # Same fp8 W1/W2 scaling as z1, but the gate weights go to fp16
    # (fp16's 10 mantissa bits beat bf16's 7; range is tiny here).
    F8 = _fp8_np()
    W1 = np.asarray(W1, np.float32)
    W2 = np.asarray(W2, np.float32)
    Wg = np.asarray(Wg, np.float32)
    b1 = np.asarray(b1, np.float32)
    b2 = np.asarray(b2, np.float32)
    bg = np.asarray(bg, np.float32)

    w1t = np.ascontiguousarray((AW1 * DT * W1[:, :D]).T).astype(F8)
    w2t = np.ascontiguousarray((AW2 * W2).T).astype(F8)
    wgt = np.ascontiguousarray(
        np.concatenate([Wg[:, :D].T, DT * Wg[:, D:].T], axis=0)
    ).astype(np.float16)

    ts = (DT * np.arange(NUM_STEPS)).astype(np.float32)
    b1r = b1.reshape(FK, P)
    wtr = np.ascontiguousarray(W1[:, D]).reshape(FK, P)
    b1e = b1r[None, :, :] + ts[:, None, None] * wtr[None, :, :]
    b1e = np.ascontiguousarray(b1e.transpose(2, 0, 1).reshape(P, NUM_STEPS * FK))
    b2c = np.ascontiguousarray(b2.reshape(FK, P).T)
    bgc = np.ascontiguousarray(bg.reshape(FK, P).T)
    return dict(w1t=w1t, w2t=w2t, wgt=wgt,
                b1e=b1e.astype(np.float32),
                b2c=b2c.astype(np.float32), bgc=bgc.astype(np.float32))


def _make_in_map_mir(x_slice, h_slice, weights):
    F8 = _fp8_np()
    xTc = np.ascontiguousarray(x_slice.T)
    hTc = np.ascontiguousarray(h_slice.T) * np.float32(1.0 / DT)
    return dict(
        hT=hTc.astype(np.float16),
        h0b=hTc.astype(F8),
        xTb=xTc.astype(np.float16),
        **weights,
    )


def _make_in_map_z1(x_slice, h_slice, weights):
    F8 = _fp8_np()
    BF = ml_dtypes.bfloat16
    xTc = np.ascontiguousarray(x_slice.T)
    hTc = np.ascontiguousarray(h_slice.T) * np.float32(1.0 / DT)
    return dict(
        hT=hTc,
        h0b=hTc.astype(F8),
        xTb=xTc.astype(BF),
        **weights,
    )


def _make_in_map(x_slice, h_slice, weights, mode=MODE):
    cdt = _cast_dt(mode)
    xs = 1.0 if mode != "fp8" else AX
    xTc = np.ascontiguousarray(x_slice.T)
    return dict(
        hT=np.ascontiguousarray(h_slice.T) * np.float32(1.0 / DT),
        xT=xTc,
        xTb=(xTc * np.float32(xs)).astype(cdt) if mode == "fp8"
        else xTc.astype(cdt),
        **weights,
    )


def kernel(current_node_features, previous_hidden_state, W1, b1, W2, b2, Wg, bg):
    from concourse.bass_utils import run_bass_kernel_spmd

    x = np.asarray(current_node_features, np.float32)
    h0 = np.asarray(previous_hidden_state, np.float32)
    if MODE == "mir":
        weights = _host_prep_mir(W1, b1, W2, b2, Wg, bg)
        mk = _make_in_map_mir
    elif MODE == "z1":
        weights = _host_prep_z1(W1, b1, W2, b2, Wg, bg)
        mk = _make_in_map_z1
    else:
        weights = _host_prep(W1, b1, W2, b2, Wg, bg)
        mk = _make_in_map

    in_maps = []
    for c in range(NCORES):
        sl = slice(c * NPC, (c + 1) * NPC)
        in_maps.append(mk(x[sl], h0[sl], weights))

    nc = _get_nc(NPC)
    trace = bool(os.environ.get("BASS_TRACE"))
    if trace:
        try:
            import antenv.axon_hooks  # noqa: F401
        except ImportError:
            # no NTFF shim installed (see test.py) -> tracing would crash
            os.environ["BASS_NEVER_TRACE"] = "1"
            trace = False
    res = run_bass_kernel_spmd(nc, in_maps, core_ids=list(range(NCORES)),
                               trace=trace)
    LAST["res"] = res

    out = np.empty((N_TOTAL, D), np.float32)
    for c in range(NCORES):
        out[c * NPC:(c + 1) * NPC] = res.results[c]["outT"].T.astype(np.float32)
    return out, out

